# revision 8
# baseline (speedup 1.0000x reference)
"""Trainium2 Bass kernel for nn_CEBlock (transformer block + attention-derived
GCN layer), data-parallel over 8 NeuronCores.

Self-contained: hardcodes all shapes; accepts the full unsharded inputs and
returns the full output.

v2: fp8(e4m3) DoubleRow matmuls (0.5 cycles/row vs bf16's 1.0) for the
accuracy-tolerant stages: QKV, V, attention logits+context, proj, MLP1.
MLP2 / GCN-support / adjacency stay bf16 (their error feeds the output
without LN damping; measured on CPU: mlp2-fp8 alone = 2.1e-2, supp-fp8
alone = 3.2e-2 rel err vs 2e-2 budget, while this combo = ~1.2e-2).
Weight sections are pre-scaled by powers of 2 on the host into e4m3 range;
descales fold into existing instructions (exp scale/bias, gelu scale,
reciprocal copy, fused scalar_tensor_tensor residual add) -- zero extra ops.

Per-core program (B_LOC=4 samples), software-pipelined in emission order:
  A1(s): load x, LN1, transpose, QKV-DR, logits-DR+exp (key-major, fp8,
         exp output shifted 2^-4 to dodge e4m3 overflow), ctx-DR,
         denominator staging + reciprocal
  A2(s): ctx normalize (broadcast grid) -> fp8 ctx, w_ts adjacency softmax
  B(s):  proj-DR + fused descale-residual, LN2, MLP1-DR(+gelu), MLP2 bf16,
         residual, GCN support + adjacency matmuls (bf16), LN3, store
Emission: A1(0), A1(1), then for s: A2(s), B(s), A1(s+2).
"""
import sys

if "/opt/trn_rl_repo" not in sys.path:
    sys.path.insert(0, "/opt/trn_rl_repo")

import math

import numpy as np
import ml_dtypes

import concourse.bacc as bacc
import concourse.mybir as mybir
import concourse.tile as tile

F32 = mybir.dt.float32
BF16 = mybir.dt.bfloat16
FP8 = mybir.dt.float8e4

# Problem constants
B, N, C = 32, 320, 768
H, HD = 12, 64
LT, LS = 64, 256
MLP_H = 3072
EPS = 1e-5
N_CORES = 8
B_LOC = B // N_CORES

P = 128
KC = C // P
FC = MLP_H // P
MC_QK = (2 * C) // P
TCH = [(0, 128), (128, 128), (256, 64)]
SH = [(0, 64), (64, 128), (192, 128)]
KSZ = [128, 128, 64]
EXP_SHIFT = 4  # probs8 = exp(logits - EXP_SHIFT*ln2); cancels in normalize

Gelu = mybir.ActivationFunctionType.Gelu
Exp = mybir.ActivationFunctionType.Exp
SUB = mybir.AluOpType.subtract
MULT = mybir.AluOpType.mult
ADD = mybir.AluOpType.add
DR = mybir.MatmulPerfMode.DoubleRow
LN2C = math.log(2.0)


def build_nc(b_loc=B_LOC, flags=None, repeat=1):
    fl = flags or {}
    aq, ak, av, k1, kp, sq, sk, sv = fl.get("scales", (0,) * 8)
    nc = bacc.Bacc("TRN2", target_bir_lowering=False, debug=True)

    x_e = nc.dram_tensor("x", (b_loc, N, C), BF16, kind="ExternalInput")
    wqkv_e = nc.dram_tensor("wqkv8", (C, 3 * C), FP8, kind="ExternalInput")
    wproj_e = nc.dram_tensor("wproj8", (C, C), FP8, kind="ExternalInput")
    w1_e = nc.dram_tensor("w18", (C, MLP_H), FP8, kind="ExternalInput")
    w2_e = nc.dram_tensor("w2", (MLP_H, C), BF16, kind="ExternalInput")
    wg_e = nc.dram_tensor("wg", (C, C), BF16, kind="ExternalInput")
    bexp_e = nc.dram_tensor("bexp", (H, C), BF16, kind="ExternalInput")
    bsel_e = nc.dram_tensor("bsel", (H, H * P), BF16, kind="ExternalInput")
    id32_e = nc.dram_tensor("id32", (P, P), F32, kind="ExternalInput")
    id16_e = nc.dram_tensor("id16", (P, P), BF16, kind="ExternalInput")
    out_e = nc.dram_tensor("out", (b_loc, N, C), F32, kind="ExternalOutput")

    has_aff3 = fl.get("has_aff3", False)
    if has_aff3:
        g3_e = nc.dram_tensor("g3v", (C,), F32, kind="ExternalInput")
        b3_e = nc.dram_tensor("b3v", (C,), F32, kind="ExternalInput")

    with tile.TileContext(nc) as tc:
        with (
            tc.tile_pool(name="wp", bufs=1) as wp,
            tc.tile_pool(name="act", bufs=1) as actp,
            tc.tile_pool(name="st", bufs=1) as stp,
            tc.tile_pool(name="ps", bufs=2, space="PSUM") as psp,
            tc.tile_pool(name="ps1", bufs=4, space="PSUM") as ps1p,
        ):
            # ---- small consts first (unblock sample-0 transposes) ----
            id32 = wp.tile([P, P], F32)
            nc.sync.dma_start(id32[:], id32_e[:])
            id16 = wp.tile([P, P], BF16)
            nc.sync.dma_start(id16[:], id16_e[:])
            bexp = wp.tile([H, KC, P], BF16)
            nc.sync.dma_start(bexp[:], bexp_e[:].rearrange("h (kc p) -> h kc p", p=P))
            bsel = wp.tile([H, H, P], BF16)
            nc.sync.dma_start(bsel[:], bsel_e[:].rearrange("a (h p) -> a h p", p=P))
            expb = wp.tile([P, 1], F32)
            nc.vector.memset(expb[:], -EXP_SHIFT * LN2C)

            # ---- x prefetch helper ----------------------------------
            def load_x(s):
                x_sb = actp.tile([P, 3, C], BF16, tag="resid", bufs=4,
                                 name=f"x_{s}")
                nc.sync.dma_start(
                    x_sb[:, 0:2, :],
                    x_e[s, 0:256, :].rearrange("(t p) c -> p t c", p=P))
                nc.sync.dma_start(x_sb[0:64, 2, :], x_e[s, 256:320, :])
                return x_sb

            # ---- helpers --------------------------------------------
            def newton_rsqrt(dst, var_ap, nf):
                pdim = dst.shape[0]
                v = stp.tile([P, 4], F32, tag="nw_v", bufs=2)
                t = stp.tile([P, 4], F32, tag="nw_t", bufs=2)
                nc.vector.tensor_scalar(
                    out=v[0:pdim, 0:nf], in0=var_ap, scalar1=EPS,
                    scalar2=None, op0=ADD)
                nc.vector.tensor_scalar(
                    out=dst[0:pdim, 0:nf].bitcast(mybir.dt.int32),
                    in0=v[0:pdim, 0:nf].bitcast(mybir.dt.int32),
                    scalar1=1, scalar2=None,
                    op0=mybir.AluOpType.logical_shift_right)
                nc.vector.tensor_scalar(
                    out=dst[0:pdim, 0:nf].bitcast(mybir.dt.int32),
                    in0=dst[0:pdim, 0:nf].bitcast(mybir.dt.int32),
                    scalar1=0x5F3759DF, scalar2=-1, op0=SUB, op1=MULT)
                for _ in range(2):
                    nc.vector.tensor_mul(out=t[0:pdim, 0:nf],
                                         in0=dst[0:pdim, 0:nf],
                                         in1=dst[0:pdim, 0:nf])
                    nc.vector.tensor_mul(out=t[0:pdim, 0:nf],
                                         in0=t[0:pdim, 0:nf],
                                         in1=v[0:pdim, 0:nf])
                    nc.vector.tensor_scalar(
                        out=t[0:pdim, 0:nf], in0=t[0:pdim, 0:nf],
                        scalar1=-0.5, scalar2=1.5, op0=MULT, op1=ADD)
                    nc.vector.tensor_mul(out=dst[0:pdim, 0:nf],
                                         in0=dst[0:pdim, 0:nf],
                                         in1=t[0:pdim, 0:nf])

            def layernorm_to(src, out_tile):
                st6 = stp.tile([P, 3, 3, 6], F32, tag="st6", bufs=2)
                mv3 = stp.tile([P, 3, 2], F32, tag="mv3", bufs=2)
                nc.vector.memset(mv3[:], 1.0)
                for tcidx, (ts, tsz) in enumerate(TCH):
                    for sub in range(3):
                        nc.vector.bn_stats(
                            out=st6[0:tsz, tcidx, sub, :],
                            in_=src[0:tsz, tcidx, sub * 256:(sub + 1) * 256])
                    nc.vector.bn_aggr(
                        out=mv3[0:tsz, tcidx, :], in_=st6[0:tsz, tcidx, :, :])
                rstd3 = stp.tile([P, 3], F32, tag="rstd3", bufs=2)
                newton_rsqrt(rstd3, mv3[:, :, 1], 3)
                for tcidx, (ts, tsz) in enumerate(TCH):
                    nc.vector.tensor_scalar(
                        out=out_tile[0:tsz, tcidx, :],
                        in0=src[0:tsz, tcidx, :],
                        scalar1=mv3[0:tsz, tcidx, 0:1],
                        scalar2=rstd3[0:tsz, tcidx:tcidx + 1],
                        op0=SUB, op1=MULT)

            def transpose_tm_to_fm(src_tm, ident, out_fm):
                nch = src_tm.shape[2] // P
                for kc in range(nch):
                    pt = ps1p.tile([P, 512], BF16, tag="one")
                    for tcidx, (ts, tsz) in enumerate(TCH):
                        nc.tensor.transpose(
                            pt[:, ts:ts + tsz],
                            src_tm[0:tsz, tcidx, kc * P:(kc + 1) * P],
                            ident[0:tsz, 0:tsz])
                    nc.any.tensor_copy(out=out_fm[:, kc, :], in_=pt[:, 0:N])

            # =========================================================
            def phase_pre(s):
                x_sb = load_x(s % b_loc)
                st = {"x": x_sb}
                xn_tm = actp.tile([P, 3, C], BF16, tag="xn", bufs=2)
                layernorm_to(x_sb, xn_tm)
                st["xn"] = xn_tm
                return st

            def phase_a1(s, st):
                x_sb, xn_tm = st["x"], st["xn"]
                xnT = actp.tile([P, KC, N], FP8, tag="actT8", bufs=2)
                transpose_tm_to_fm(xn_tm, id16, xnT)

                # q/k feature-major via fp8 DoubleRow; PSUM rows split into
                # the [32-part, head, hd-half] layout the logits-DR needs.
                q8 = actp.tile([32, H, 2, N], FP8, tag="q8")
                k8 = actp.tile([32, H, 2, N], FP8, tag="k8")
                for mc in range(MC_QK):
                    pq = ps1p.tile([P, 512], F32, tag="one")
                    for g in range(KC // 2):
                        nc.tensor.matmul(
                            pq[:, 0:N],
                            lhsT=wqkv[:, 2 * g:2 * g + 2, mc * P:(mc + 1) * P],
                            rhs=xnT[:, 2 * g:2 * g + 2, :],
                            start=(g == 0), stop=(g == KC // 2 - 1),
                            perf_mode=DR)
                    dst = q8 if mc < KC else k8
                    qdesc = float(2.0 ** ((sq - aq) if mc < KC else (sk - ak)))
                    h0 = (mc % KC) * 2
                    for hh in range(2):
                        for ii in range(2):
                            base = hh * 64 + ii * 32
                            if hh == 0:
                                nc.vector.tensor_scalar(
                                    out=dst[0:32, h0 + hh, ii, :],
                                    in0=pq[base:base + 32, 0:N],
                                    scalar1=qdesc, scalar2=None, op0=MULT)
                            else:
                                nc.scalar.mul(
                                    dst[0:32, h0 + hh, ii, :],
                                    pq[base:base + 32, 0:N], qdesc)

                # HD+4: dual-fp8 Ldweights needs 16B-aligned pair stride (12*68=816)
                v_sb = actp.tile([P, 3, H, HD + 4], FP8, tag="vsm")
                for tcidx, (ts, tsz) in enumerate(TCH):
                    for half in range(2):
                        pv = ps1p.tile([P, 512], F32, tag="one")
                        for g in range(KC // 2):
                            nc.tensor.matmul(
                                pv[0:tsz, 0:384],
                                lhsT=xnT[:, 2 * g:2 * g + 2, ts:ts + tsz],
                                rhs=wqkv[:, 2 * g:2 * g + 2,
                                         2 * C + half * 384:2 * C + (half + 1) * 384],
                                start=(g == 0), stop=(g == KC // 2 - 1),
                                perf_mode=DR)
                        nc.any.tensor_scalar(
                            out=v_sb[0:tsz, tcidx, half * 6:(half + 1) * 6, 0:HD],
                            in0=pv[0:tsz, 0:384].rearrange(
                                "p (h d) -> p h d", d=HD),
                            scalar1=float(2.0 ** (sv - av)), scalar2=None,
                            op0=MULT)
                    nc.vector.memset(v_sb[0:tsz, tcidx, :, HD:HD + 1], 1.0)

                probs = actp.tile([P, 3, H, N], FP8, tag="big")
                exp_scale = float(2.0 ** (-(sq + sk)))
                for j, kz in enumerate(KSZ):
                    for g in range(H // 2):
                        pl = psp.tile([P, 2, 512], F32, tag="pair")
                        for hh in range(2):
                            h = 2 * g + hh
                            nc.tensor.matmul(
                                pl[0:kz, hh, 0:N],
                                lhsT=k8[0:32, h, :,
                                        TCH[j][0]:TCH[j][0] + kz],
                                rhs=q8[0:32, h, :, :],
                                start=True, stop=True, perf_mode=DR)
                        nc.scalar.activation(
                            out=probs[0:kz, j, 2 * g:2 * g + 2, :],
                            in_=pl[0:kz, :, 0:N], func=Exp,
                            scale=exp_scale, bias=expb[0:kz, 0:1])

                # small template-column copy so probs can be released at
                # the end of A1 (w_ts consumes this in A2 instead).
                pw_sb = actp.tile([P, 2, H, LT], BF16, tag="pwts", bufs=2)
                nc.vector.tensor_copy(
                    out=pw_sb[:, 0, :, :], in_=probs[:, 1, :, 0:LT])
                nc.vector.tensor_copy(
                    out=pw_sb[0:64, 1, :, :], in_=probs[64:128, 0, :, 0:LT])
                nc.vector.tensor_copy(
                    out=pw_sb[64:128, 1, :, :], in_=probs[0:64, 2, :, 0:LT])

                ctx = actp.tile([P, KC, N], BF16, tag="ctx", bufs=2)
                dstage = stp.tile([H, N], F32, tag="dstage")
                for h in range(H):
                    pc = ps1p.tile([HD + 1, 512], F32, tag="one")
                    nc.tensor.matmul(
                        pc[:, 0:N],
                        lhsT=v_sb[:, 0:2, h, 0:HD + 1],
                        rhs=probs[:, 0:2, h, :],
                        start=True, stop=False, perf_mode=DR)
                    nc.tensor.matmul(
                        pc[:, 0:N],
                        lhsT=v_sb[0:64, 2, h, 0:HD + 1],
                        rhs=probs[0:64, 2, h, :],
                        start=False, stop=True)
                    nc.vector.tensor_copy(
                        out=ctx[(h % 2) * HD:(h % 2) * HD + HD, h // 2, :],
                        in_=pc[0:HD, 0:N])
                    d64 = stp.tile([HD + 1, N], F32, tag="d64", bufs=2)
                    nc.any.tensor_copy(
                        out=d64[HD:HD + 1, :], in_=pc[HD:HD + 1, 0:N])
                    nc.sync.dma_start(dstage[h:h + 1, :], d64[HD:HD + 1, :])
                recf = stp.tile([H, N], F32, tag="recf")
                nc.vector.reciprocal(out=recf[:], in_=dstage[:])
                recip = stp.tile([H, N], BF16, tag="recip", bufs=2)
                nc.vector.tensor_scalar(
                    out=recip[:], in0=recf[:], scalar1=float(2.0 ** (-sv)),
                    scalar2=None, op0=MULT)
                st.update(ctx=ctx, pw_sb=pw_sb, recip=recip)
                return st

            def phase_a2_grid(s, st):
                ctx, recip, pw_sb = st["ctx"], st["recip"], st["pw_sb"]
                # normalize ctx via broadcast grid; fp8 out for proj-DR
                ctx8 = actp.tile([P, KC, N], FP8, tag="ctx8", bufs=2)
                for kc in range(KC):
                    pg = ps1p.tile([P, 512], F32, tag="one")
                    nc.tensor.matmul(pg[:, 0:N], lhsT=bexp[:, kc, :],
                                     rhs=recip[:], start=True, stop=True)
                    nc.vector.tensor_mul(
                        out=ctx8[:, kc, :], in0=ctx[:, kc, :], in1=pg[:, 0:N])
                # w_ts pieces (DVE work runs under the proj matmuls)
                prk = psp.tile([P, 2, 512], F32, tag="pair")
                for h in range(H):
                    fo = h * HD
                    nc.tensor.matmul(
                        prk[:, fo // 512, fo % 512:fo % 512 + HD],
                        lhsT=bsel[:, h, :], rhs=recip[:, 0:LT],
                        start=True, stop=True)
                prk_v = prk[:].rearrange("p a b -> p (a b)")[:, 0:H * HD] \
                    .rearrange("p (h i) -> p h i", i=HD)
                wtmp = stp.tile([P, H, LT], BF16, tag="wtmp")
                wpiece = stp.tile([P, 3, LT], F32, tag="wpiece")
                for j, (slot, po, kz) in enumerate(
                        [(1, 0, 64), (0, 0, 128), (1, 64, 64)]):
                    nc.vector.tensor_tensor(
                        out=wtmp[po:po + kz, :, :],
                        in0=pw_sb[po:po + kz, slot, :, :],
                        in1=prk_v[po:po + kz, :, :], op=MULT)
                    nc.vector.tensor_reduce(
                        out=wpiece[0:kz, j, :],
                        in_=wtmp[po:po + kz, :, :].rearrange("p h i -> p i h"),
                        axis=mybir.AxisListType.X, op=ADD)
                st["wpiece"] = wpiece
                st["ctx8"] = ctx8

            def phase_a2_wts(s, st):
                wpiece = st["wpiece"]
                pw = ps1p.tile([64, 512], F32, tag="one")
                nc.tensor.transpose(pw[:, 0:64], wpiece[0:64, 0, :],
                                    id32[0:64, 0:64])
                nc.tensor.transpose(pw[:, 64:192], wpiece[0:128, 1, :],
                                    id32[:, :])
                nc.tensor.transpose(pw[:, 192:256], wpiece[0:64, 2, :],
                                    id32[0:64, 0:64])
                wts_e = stp.tile([LT, LS], BF16, tag="wts_e")
                wden = stp.tile([LT, 1], F32, tag="wden", bufs=2)
                # w_raw carries 2^-av (from recip); undo it inside the exp
                nc.scalar.activation(out=wts_e[:], in_=pw[:, 0:LS], func=Exp,
                                     scale=float(2.0 ** sv) / H,
                                     accum_out=wden[:])
                wrec = stp.tile([LT, 1], F32, tag="wrec", bufs=2)
                nc.vector.reciprocal(out=wrec[:], in_=wden[:])
                wts = stp.tile([LT, LS], BF16, tag="wts", bufs=2)
                nc.vector.tensor_scalar_mul(wts[:], wts_e[:], wrec[:])
                wtsT = stp.tile([P, 2, LT], BF16, tag="wtsT", bufs=2)
                for mm in range(2):
                    pwt = ps1p.tile([P, 512], BF16, tag="one")
                    nc.tensor.transpose(pwt[:, 0:LT],
                                        wts[:, mm * P:(mm + 1) * P],
                                        id16[0:LT, 0:LT])
                    nc.any.tensor_copy(out=wtsT[:, mm, :], in_=pwt[:, 0:LT])
                st.update(wts=wts, wtsT=wtsT)

            def phase_b_proj(s, st):
                x_sb, ctx8 = st["x"], st["ctx8"]
                x1_sb = actp.tile([P, 3, C], BF16, tag="resid", bufs=4)
                st6 = stp.tile([P, 3, 3, 6], F32, tag="st6", bufs=2)
                mv3 = stp.tile([P, 3, 2], F32, tag="mv3", bufs=2)
                nc.vector.memset(mv3[:], 1.0)
                pdesc = float(2.0 ** (-kp))
                for tcidx, (ts, tsz) in enumerate(TCH):
                    for half in range(2):
                        pp = ps1p.tile([P, 512], F32, tag="one")
                        for g in range(KC // 2):
                            nc.tensor.matmul(
                                pp[0:tsz, 0:384],
                                lhsT=ctx8[:, 2 * g:2 * g + 2, ts:ts + tsz],
                                rhs=wproj[:, 2 * g:2 * g + 2,
                                          half * 384:(half + 1) * 384],
                                start=(g == 0), stop=(g == KC // 2 - 1),
                                perf_mode=DR)
                        nc.vector.scalar_tensor_tensor(
                            out=x1_sb[0:tsz, tcidx, half * 384:(half + 1) * 384],
                            in0=pp[0:tsz, 0:384], scalar=pdesc,
                            in1=x_sb[0:tsz, tcidx, half * 384:(half + 1) * 384],
                            op0=MULT, op1=ADD)
                    # LN2 stats interleaved per chunk
                    for sub in range(3):
                        nc.vector.bn_stats(
                            out=st6[0:tsz, tcidx, sub, :],
                            in_=x1_sb[0:tsz, tcidx, sub * 256:(sub + 1) * 256])
                    nc.vector.bn_aggr(
                        out=mv3[0:tsz, tcidx, :], in_=st6[0:tsz, tcidx, :, :])
                rstd3 = stp.tile([P, 3], F32, tag="rstd3", bufs=2)
                newton_rsqrt(rstd3, mv3[:, :, 1], 3)
                xn2_tm = actp.tile([P, 3, C], BF16, tag="xn", bufs=2)
                for tcidx, (ts, tsz) in enumerate(TCH):
                    nc.vector.tensor_scalar(
                        out=xn2_tm[0:tsz, tcidx, :],
                        in0=x1_sb[0:tsz, tcidx, :],
                        scalar1=mv3[0:tsz, tcidx, 0:1],
                        scalar2=rstd3[0:tsz, tcidx:tcidx + 1],
                        op0=SUB, op1=MULT)
                st.update(x1=x1_sb, xn2=xn2_tm)

            def phase_b_rest(s, st):
                x1_sb, xn2_tm = st["x1"], st["xn2"]
                wts, wtsT = st["wts"], st["wtsT"]
                xn2T = actp.tile([P, KC, N], FP8, tag="actT8", bufs=2)
                transpose_tm_to_fm(xn2_tm, id16, xn2T)

                h_sb = actp.tile([P, FC, N], BF16, tag="big")
                g1desc = float(2.0 ** (-k1))
                for g in range(FC // 2):
                    ph = psp.tile([P, 2, 512], F32, tag="pair")
                    for ff in range(2):
                        fcl = 2 * g + ff
                        for gg in range(KC // 2):
                            nc.tensor.matmul(
                                ph[:, ff, 0:N],
                                lhsT=w1[:, 2 * gg:2 * gg + 2,
                                        fcl * P:(fcl + 1) * P],
                                rhs=xn2T[:, 2 * gg:2 * gg + 2, :],
                                start=(gg == 0), stop=(gg == KC // 2 - 1),
                                perf_mode=DR)
                    nc.scalar.activation(
                        out=h_sb[:, 2 * g:2 * g + 2, :],
                        in_=ph[:, :, 0:N], func=Gelu, scale=g1desc)

                # MLP2 feature-major (bf16), then transpose back
                m2T = actp.tile([P, KC, N], BF16, tag="actT", bufs=2)
                for cc in range(KC):
                    pm = ps1p.tile([P, 512], F32, tag="one")
                    for fc in range(FC):
                        nc.tensor.matmul(
                            pm[:, 0:N],
                            lhsT=w2[:, fc, cc * P:(cc + 1) * P],
                            rhs=h_sb[:, fc, :],
                            start=(fc == 0), stop=(fc == FC - 1))
                    nc.any.tensor_copy(out=m2T[:, cc, :], in_=pm[:, 0:N])
                x2_sb = actp.tile([P, 3, C], BF16, tag="resid", bufs=4)
                for tcidx, (ts, tsz) in enumerate(TCH):
                    pa = ps1p.tile([P, 512], BF16, tag="one")
                    pb = ps1p.tile([P, 512], BF16, tag="one")
                    for cc in range(KC):
                        dst = pa[0:tsz, cc * P:(cc + 1) * P] if cc < 4 else \
                            pb[0:tsz, (cc - 4) * P:(cc - 3) * P]
                        nc.tensor.transpose(
                            dst, m2T[:, cc, ts:ts + tsz], id16[:, :])
                    nc.vector.tensor_add(
                        out=x2_sb[0:tsz, tcidx, 0:512],
                        in0=x1_sb[0:tsz, tcidx, 0:512], in1=pa[0:tsz, 0:512])
                    nc.vector.tensor_add(
                        out=x2_sb[0:tsz, tcidx, 512:768],
                        in0=x1_sb[0:tsz, tcidx, 512:768], in1=pb[0:tsz, 0:256])

                x2T = actp.tile([P, KC, N], BF16, tag="actT", bufs=2)
                transpose_tm_to_fm(x2_sb, id16, x2T)
                supp = actp.tile([P, 3, C], BF16, tag="vsm")
                for scidx, (ss, ssz) in enumerate(SH):
                    for half in range(2):
                        psu = ps1p.tile([P, 512], F32, tag="one")
                        for kc in range(KC):
                            nc.tensor.matmul(
                                psu[0:ssz, 0:384],
                                lhsT=x2T[:, kc, ss:ss + ssz],
                                rhs=wg[:, kc, half * 384:(half + 1) * 384],
                                start=(kc == 0), stop=(kc == KC - 1))
                        nc.any.tensor_copy(
                            out=supp[0:ssz, scidx, half * 384:(half + 1) * 384],
                            in_=psu[0:ssz, 0:384])

                for scidx, (ss, ssz) in enumerate(SH):
                    y_sb = stp.tile([P, C], F32, tag="ych", bufs=2)
                    for half in range(2):
                        py = ps1p.tile([P, 512], F32, tag="one")
                        if scidx == 0:
                            for mm in range(2):
                                nc.tensor.matmul(
                                    py[0:64, 0:384],
                                    lhsT=wtsT[:, mm, :],
                                    rhs=supp[:, mm + 1,
                                             half * 384:(half + 1) * 384],
                                    start=(mm == 0), stop=(mm == 1))
                        else:
                            nc.tensor.matmul(
                                py[0:ssz, 0:384],
                                lhsT=wts[:, (scidx - 1) * P:scidx * P],
                                rhs=supp[0:64, 0, half * 384:(half + 1) * 384],
                                start=True, stop=True)
                        nc.any.tensor_copy(
                            out=y_sb[0:ssz, half * 384:(half + 1) * 384],
                            in_=py[0:ssz, 0:384])
                    st6y = stp.tile([P, 3, 6], F32, tag="st6y", bufs=2)
                    for sub in range(3):
                        nc.vector.bn_stats(
                            out=st6y[0:ssz, sub, :],
                            in_=y_sb[0:ssz, sub * 256:(sub + 1) * 256])
                    mvy = stp.tile([P, 2], F32, tag="mvy", bufs=2)
                    nc.vector.bn_aggr(out=mvy[0:ssz, :], in_=st6y[0:ssz, :, :])
                    rstdy = stp.tile([P, 1], F32, tag="rstdy", bufs=2)
                    newton_rsqrt(rstdy[0:ssz], mvy[0:ssz, 1:2], 1)
                    yfin = stp.tile([P, C], F32, tag="yfin")
                    nc.vector.tensor_scalar(
                        out=yfin[0:ssz, :], in0=y_sb[0:ssz, :],
                        scalar1=mvy[0:ssz, 0:1], scalar2=rstdy[0:ssz, 0:1],
                        op0=SUB, op1=MULT)
                    if has_aff3:
                        nc.vector.tensor_mul(out=yfin[0:ssz, :],
                                             in0=yfin[0:ssz, :],
                                             in1=g3b[0:ssz, :])
                        nc.vector.tensor_add(out=yfin[0:ssz, :],
                                             in0=yfin[0:ssz, :],
                                             in1=b3b[0:ssz, :])
                    nc.vector.tensor_add(out=yfin[0:ssz, :],
                                         in0=yfin[0:ssz, :],
                                         in1=y_sb[0:ssz, :])
                    nc.sync.dma_start(out_e[s, ss:ss + ssz, :], yfin[0:ssz, :])

            # ---- software-pipelined emission ------------------------
            total = b_loc * repeat
            states = {}
            states[0] = phase_pre(0)
            if total > 1:
                states[1] = phase_pre(1)
            # ---- weights (in order of first use) --------------------
            wqkv = wp.tile([P, KC, 3 * C], FP8)
            for kc in range(KC):
                nc.sync.dma_start(
                    wqkv[:, kc, :],
                    wqkv_e[kc * P:(kc + 1) * P, :])
            wproj = wp.tile([P, KC, C], FP8)
            for kc in range(KC):
                nc.sync.dma_start(
                    wproj[:, kc, :], wproj_e[kc * P:(kc + 1) * P, :])
            w1 = wp.tile([P, KC, MLP_H], FP8)
            for kc in range(KC):
                nc.sync.dma_start(
                    w1[:, kc, :], w1_e[kc * P:(kc + 1) * P, :])
            w2 = wp.tile([P, FC, C], BF16)
            for fc in range(0, FC, 4):
                nc.sync.dma_start(
                    w2[:, fc:fc + 4, :],
                    w2_e[fc * P:(fc + 4) * P, :].rearrange(
                        "(fc p) m -> p fc m", p=P))
            wg = wp.tile([P, KC, C], BF16)
            for kc in range(KC):
                nc.sync.dma_start(
                    wg[:, kc, :], wg_e[kc * P:(kc + 1) * P, :])
            if has_aff3:
                g3b = wp.tile([P, C], F32)
                nc.sync.dma_start(g3b[:], g3_e[None, :].to_broadcast((P, C)))
                b3b = wp.tile([P, C], F32)
                nc.sync.dma_start(b3b[:], b3_e[None, :].to_broadcast((P, C)))

            phase_a1(0, states[0])
            if total > 1:
                phase_a1(1, states[1])
            for i in range(total):
                phase_a2_grid(i, states[i])
                phase_b_proj(i % b_loc, states[i])
                phase_a2_wts(i, states[i])
                if i + 2 < total:
                    states[i + 2] = phase_pre(i + 2)
                phase_b_rest(i % b_loc, states[i])
                del states[i]
                if i + 2 < total:
                    phase_a1(i + 2, states[i + 2])

    nc.finalize()
    return nc


def _pow2_scale(w):
    """power-of-2 exponent putting max|w| around 224 (e4m3 max 448)."""
    m = float(np.abs(w).max())
    if m <= 0:
        return 0
    return int(math.floor(math.log2(224.0 / m)))


def _preprocess(inputs):
    f32 = np.float32
    g1 = np.asarray(inputs["g1"], f32)
    b1 = np.asarray(inputs["b1"], f32)
    g2 = np.asarray(inputs["g2"], f32)
    b2 = np.asarray(inputs["b2"], f32)
    wqkv = np.asarray(inputs["Wqkv"], f32)
    w1 = np.asarray(inputs["W1"], f32)

    wqkv_eff = g1[:, None] * wqkv
    bqkv_eff = b1 @ wqkv
    scale = HD ** (-0.5)
    wqkv_eff[:, 0:C] *= scale
    bqkv_eff[0:C] *= scale

    w1_eff = g2[:, None] * w1
    bm1_eff = np.asarray(inputs["bm1"], f32) + b2 @ w1

    wproj = np.asarray(inputs["Wproj"], f32)
    bproj = np.asarray(inputs["bproj"], f32)
    bm2 = np.asarray(inputs["bm2"], f32)
    bg = np.asarray(inputs["bg"], f32)
    g3 = np.asarray(inputs["g3"], f32)
    b3 = np.asarray(inputs["b3"], f32)

    has_bias = any(
        np.abs(v).max() > 0 for v in (bqkv_eff, bm1_eff, bproj, bm2, bg))
    assert not has_bias, "fp8 kernel assumes zero biases"

    aq = _pow2_scale(wqkv_eff[:, 0:C])
    ak = _pow2_scale(wqkv_eff[:, C:2 * C])
    av = _pow2_scale(wqkv_eff[:, 2 * C:])
    k1 = _pow2_scale(w1_eff)
    kp = _pow2_scale(wproj)

    def out_scale(w):
        bound = math.sqrt(C) * float(
            np.sqrt((w.astype(np.float64) ** 2).sum(axis=0)).max())
        return int(math.floor(math.log2(224.0 / bound)))

    sq = out_scale(wqkv_eff[:, 0:C])
    sk = out_scale(wqkv_eff[:, C:2 * C])
    sv = out_scale(wqkv_eff[:, 2 * C:])

    wqkv_s = wqkv_eff.copy()
    wqkv_s[:, 0:C] *= 2.0 ** aq
    wqkv_s[:, C:2 * C] *= 2.0 ** ak
    wqkv_s[:, 2 * C:] *= 2.0 ** av

    bexp = np.zeros((H, C), f32)
    for h in range(H):
        bexp[h, h * HD:(h + 1) * HD] = 1.0
    bsel = np.kron(np.eye(H, dtype=f32), np.ones((1, P), f32))

    bf = ml_dtypes.bfloat16
    f8 = ml_dtypes.float8_e4m3
    wm = {
        "wqkv8": wqkv_s.astype(f8),
        "wproj8": (wproj * 2.0 ** kp).astype(f8),
        "w18": (w1_eff * 2.0 ** k1).astype(f8),
        "w2": np.asarray(inputs["W2"], f32).astype(bf),
        "wg": np.asarray(inputs["Wg"], f32).astype(bf),
        "bexp": bexp.astype(bf),
        "bsel": bsel.astype(bf),
        "id32": np.eye(P, dtype=f32),
        "id16": np.eye(P, dtype=f32).astype(bf),
    }
    flags = {"scales": (aq, ak, av, k1, kp, sq, sk, sv)}
    has_aff3 = bool(np.abs(g3 - 1.0).max() > 0 or np.abs(b3).max() > 0)
    flags["has_aff3"] = has_aff3
    if has_aff3:
        wm["g3v"] = g3
        wm["b3v"] = b3
    return wm, flags


_CACHE = {}


def _get_nc(flags, b_loc=B_LOC):
    key = (tuple(sorted((k, v) for k, v in flags.items())), b_loc)
    if key not in _CACHE:
        _CACHE[key] = build_nc(b_loc=b_loc, flags=flags)
    return _CACHE[key]


def kernel(**inputs) -> np.ndarray:
    from concourse.bass_utils import run_bass_kernel_spmd

    x = np.ascontiguousarray(
        np.asarray(inputs["x"], np.float32)).astype(ml_dtypes.bfloat16)
    wm, flags = _preprocess(inputs)
    nc = _get_nc(flags)

    in_maps = []
    for i in range(N_CORES):
        m = dict(wm)
        m["x"] = np.ascontiguousarray(x[i * B_LOC:(i + 1) * B_LOC])
        in_maps.append(m)

    res = run_bass_kernel_spmd(nc, in_maps, core_ids=list(range(N_CORES)))
    out = np.concatenate([res.results[i]["out"] for i in range(N_CORES)], axis=0)
    return out.astype(np.float32)


# revision 9
# speedup vs baseline: 1.1476x; 1.1476x over previous
"""Trainium2 Bass kernel for nn_CEBlock (transformer block + attention-derived
GCN layer), data-parallel over 8 NeuronCores.

Self-contained: hardcodes all shapes; accepts the full unsharded inputs and
returns the full output.

v2: fp8(e4m3) DoubleRow matmuls (0.5 cycles/row vs bf16's 1.0) for the
accuracy-tolerant stages: QKV, V, attention logits+context, proj, MLP1.
MLP2 / GCN-support / adjacency stay bf16 (their error feeds the output
without LN damping; measured on CPU: mlp2-fp8 alone = 2.1e-2, supp-fp8
alone = 3.2e-2 rel err vs 2e-2 budget, while this combo = ~1.2e-2).
Weight sections are pre-scaled by powers of 2 on the host into e4m3 range;
descales fold into existing instructions (exp scale/bias, gelu scale,
reciprocal copy, fused scalar_tensor_tensor residual add) -- zero extra ops.

Per-core program (B_LOC=4 samples), software-pipelined in emission order:
  A1(s): load x, LN1, transpose, QKV-DR, logits-DR+exp (key-major, fp8,
         exp output shifted 2^-4 to dodge e4m3 overflow), ctx-DR,
         denominator staging + reciprocal
  A2(s): ctx normalize (broadcast grid) -> fp8 ctx, w_ts adjacency softmax
  B(s):  proj-DR + fused descale-residual, LN2, MLP1-DR(+gelu), MLP2 bf16,
         residual, GCN support + adjacency matmuls (bf16), LN3, store
Emission: A1(0), A1(1), then for s: A2(s), B(s), A1(s+2).
"""
import sys

if "/opt/trn_rl_repo" not in sys.path:
    sys.path.insert(0, "/opt/trn_rl_repo")

import math

import numpy as np
import ml_dtypes

import concourse.bacc as bacc
import concourse.mybir as mybir
import concourse.tile as tile

F32 = mybir.dt.float32
BF16 = mybir.dt.bfloat16
FP8 = mybir.dt.float8e4

# Problem constants
B, N, C = 32, 320, 768
H, HD = 12, 64
LT, LS = 64, 256
MLP_H = 3072
EPS = 1e-5
N_CORES = 8
B_LOC = B // N_CORES

P = 128
KC = C // P
FC = MLP_H // P
MC_QK = (2 * C) // P
TCH = [(0, 128), (128, 128), (256, 64)]
SH = [(0, 64), (64, 128), (192, 128)]
KSZ = [128, 128, 64]
EXP_SHIFT = 4  # probs8 = exp(logits - EXP_SHIFT*ln2); cancels in normalize

Gelu = mybir.ActivationFunctionType.Gelu
Exp = mybir.ActivationFunctionType.Exp
SUB = mybir.AluOpType.subtract
MULT = mybir.AluOpType.mult
ADD = mybir.AluOpType.add
DR = mybir.MatmulPerfMode.DoubleRow
LN2C = math.log(2.0)


def build_nc(b_loc=B_LOC, flags=None, repeat=1):
    fl = flags or {}
    aq, ak, av, k1, kp, sv = fl.get("scales", (0,) * 6)
    nc = bacc.Bacc("TRN2", target_bir_lowering=False, debug=True)

    x_e = nc.dram_tensor("x", (b_loc, N, C), BF16, kind="ExternalInput")
    wqkv_e = nc.dram_tensor("wqkv8", (C, 3 * C), FP8, kind="ExternalInput")
    wproj_e = nc.dram_tensor("wproj8", (C, C), FP8, kind="ExternalInput")
    w1_e = nc.dram_tensor("w18", (C, MLP_H), FP8, kind="ExternalInput")
    w2_e = nc.dram_tensor("w2", (MLP_H, C), BF16, kind="ExternalInput")
    wg_e = nc.dram_tensor("wg", (C, C), BF16, kind="ExternalInput")
    bexp_e = nc.dram_tensor("bexp", (H, C), BF16, kind="ExternalInput")
    bsel_e = nc.dram_tensor("bsel", (H, H * P), BF16, kind="ExternalInput")
    id32_e = nc.dram_tensor("id32", (P, P), F32, kind="ExternalInput")
    id16_e = nc.dram_tensor("id16", (P, P), BF16, kind="ExternalInput")
    out_e = nc.dram_tensor("out", (b_loc, N, C), F32, kind="ExternalOutput")

    has_aff3 = fl.get("has_aff3", False)
    if has_aff3:
        g3_e = nc.dram_tensor("g3v", (C,), F32, kind="ExternalInput")
        b3_e = nc.dram_tensor("b3v", (C,), F32, kind="ExternalInput")

    with tile.TileContext(nc) as tc:
        with (
            tc.tile_pool(name="wp", bufs=1) as wp,
            tc.tile_pool(name="act", bufs=1) as actp,
            tc.tile_pool(name="st", bufs=1) as stp,
            tc.tile_pool(name="ps", bufs=2, space="PSUM") as psp,
            tc.tile_pool(name="ps1", bufs=4, space="PSUM") as ps1p,
        ):
            # ---- small consts first (unblock sample-0 transposes) ----
            id32 = wp.tile([P, P], F32)
            nc.sync.dma_start(id32[:], id32_e[:])
            id16 = wp.tile([P, P], BF16)
            nc.sync.dma_start(id16[:], id16_e[:])
            bexp = wp.tile([H, KC, P], BF16)
            nc.sync.dma_start(bexp[:], bexp_e[:].rearrange("h (kc p) -> h kc p", p=P))
            bsel = wp.tile([H, H, P], BF16)
            nc.sync.dma_start(bsel[:], bsel_e[:].rearrange("a (h p) -> a h p", p=P))
            expb = wp.tile([P, 1], F32)
            nc.vector.memset(expb[:], -EXP_SHIFT * LN2C)

            # ---- x prefetch helper ----------------------------------
            def load_x(s):
                x_sb = actp.tile([P, 3, C], BF16, tag="resid", bufs=4,
                                 name=f"x_{s}")
                nc.sync.dma_start(
                    x_sb[:, 0:2, :],
                    x_e[s, 0:256, :].rearrange("(t p) c -> p t c", p=P))
                nc.sync.dma_start(x_sb[0:64, 2, :], x_e[s, 256:320, :])
                return x_sb

            # ---- helpers --------------------------------------------
            def newton_rsqrt(dst, var_ap, nf):
                pdim = dst.shape[0]
                v = stp.tile([P, 4], F32, tag="nw_v", bufs=2)
                t = stp.tile([P, 4], F32, tag="nw_t", bufs=2)
                nc.vector.tensor_scalar(
                    out=v[0:pdim, 0:nf], in0=var_ap, scalar1=EPS,
                    scalar2=None, op0=ADD)
                nc.vector.tensor_scalar(
                    out=dst[0:pdim, 0:nf].bitcast(mybir.dt.int32),
                    in0=v[0:pdim, 0:nf].bitcast(mybir.dt.int32),
                    scalar1=1, scalar2=None,
                    op0=mybir.AluOpType.logical_shift_right)
                nc.vector.tensor_scalar(
                    out=dst[0:pdim, 0:nf].bitcast(mybir.dt.int32),
                    in0=dst[0:pdim, 0:nf].bitcast(mybir.dt.int32),
                    scalar1=0x5F3759DF, scalar2=-1, op0=SUB, op1=MULT)
                for _ in range(2):
                    nc.vector.tensor_mul(out=t[0:pdim, 0:nf],
                                         in0=dst[0:pdim, 0:nf],
                                         in1=dst[0:pdim, 0:nf])
                    nc.vector.tensor_mul(out=t[0:pdim, 0:nf],
                                         in0=t[0:pdim, 0:nf],
                                         in1=v[0:pdim, 0:nf])
                    nc.vector.tensor_scalar(
                        out=t[0:pdim, 0:nf], in0=t[0:pdim, 0:nf],
                        scalar1=-0.5, scalar2=1.5, op0=MULT, op1=ADD)
                    nc.vector.tensor_mul(out=dst[0:pdim, 0:nf],
                                         in0=dst[0:pdim, 0:nf],
                                         in1=t[0:pdim, 0:nf])

            def layernorm_to(src, out_tile):
                st6 = stp.tile([P, 3, 3, 6], F32, tag="st6", bufs=2)
                mv3 = stp.tile([P, 3, 2], F32, tag="mv3", bufs=2)
                nc.vector.memset(mv3[:], 1.0)
                for tcidx, (ts, tsz) in enumerate(TCH):
                    for sub in range(3):
                        nc.vector.bn_stats(
                            out=st6[0:tsz, tcidx, sub, :],
                            in_=src[0:tsz, tcidx, sub * 256:(sub + 1) * 256])
                    nc.vector.bn_aggr(
                        out=mv3[0:tsz, tcidx, :], in_=st6[0:tsz, tcidx, :, :])
                rstd3 = stp.tile([P, 3], F32, tag="rstd3", bufs=2)
                newton_rsqrt(rstd3, mv3[:, :, 1], 3)
                for tcidx, (ts, tsz) in enumerate(TCH):
                    nc.vector.tensor_scalar(
                        out=out_tile[0:tsz, tcidx, :],
                        in0=src[0:tsz, tcidx, :],
                        scalar1=mv3[0:tsz, tcidx, 0:1],
                        scalar2=rstd3[0:tsz, tcidx:tcidx + 1],
                        op0=SUB, op1=MULT)

            def transpose_tm_to_fm(src_tm, ident, out_fm):
                nch = src_tm.shape[2] // P
                for kc in range(nch):
                    pt = ps1p.tile([P, 512], BF16, tag="one")
                    for tcidx, (ts, tsz) in enumerate(TCH):
                        nc.tensor.transpose(
                            pt[:, ts:ts + tsz],
                            src_tm[0:tsz, tcidx, kc * P:(kc + 1) * P],
                            ident[0:tsz, 0:tsz])
                    nc.any.tensor_copy(out=out_fm[:, kc, :], in_=pt[:, 0:N])

            # =========================================================
            def phase_pre(s):
                x_sb = load_x(s % b_loc)
                st = {"x": x_sb}
                xn_tm = actp.tile([P, 3, C], BF16, tag="xn", bufs=2)
                layernorm_to(x_sb, xn_tm)
                st["xn"] = xn_tm
                return st

            def phase_a1(s, st):
                x_sb, xn_tm = st["x"], st["xn"]
                xnT = actp.tile([P, KC, N], FP8, tag="actT8", bufs=2)
                transpose_tm_to_fm(xn_tm, id16, xnT)

                qk = actp.tile([P, MC_QK, N], BF16, tag="qk")
                for mc in range(MC_QK):
                    pq = ps1p.tile([P, 512], F32, tag="one")
                    for g in range(KC // 2):
                        nc.tensor.matmul(
                            pq[:, 0:N],
                            lhsT=wqkv[:, 2 * g:2 * g + 2, mc * P:(mc + 1) * P],
                            rhs=xnT[:, 2 * g:2 * g + 2, :],
                            start=(g == 0), stop=(g == KC // 2 - 1),
                            perf_mode=DR)
                    nc.any.tensor_copy(out=qk[:, mc, :], in_=pq[:, 0:N])

                # HD+4: dual-fp8 Ldweights needs 16B-aligned pair stride (12*68=816)
                v_sb = actp.tile([P, 3, H, HD + 4], FP8, tag="vsm")
                for tcidx, (ts, tsz) in enumerate(TCH):
                    for half in range(2):
                        pv = ps1p.tile([P, 512], F32, tag="one")
                        for g in range(KC // 2):
                            nc.tensor.matmul(
                                pv[0:tsz, 0:384],
                                lhsT=xnT[:, 2 * g:2 * g + 2, ts:ts + tsz],
                                rhs=wqkv[:, 2 * g:2 * g + 2,
                                         2 * C + half * 384:2 * C + (half + 1) * 384],
                                start=(g == 0), stop=(g == KC // 2 - 1),
                                perf_mode=DR)
                        nc.any.tensor_scalar(
                            out=v_sb[0:tsz, tcidx, half * 6:(half + 1) * 6, 0:HD],
                            in0=pv[0:tsz, 0:384].rearrange(
                                "p (h d) -> p h d", d=HD),
                            scalar1=float(2.0 ** (sv - av)), scalar2=None,
                            op0=MULT)
                    nc.vector.memset(v_sb[0:tsz, tcidx, :, HD:HD + 1], 1.0)

                probs = actp.tile([P, 3, H, N], FP8, tag="big")
                exp_scale = float(2.0 ** (-(aq + ak)))
                for j, kz in enumerate(KSZ):
                    for g in range(H // 2):
                        pl = psp.tile([P, 2, 512], F32, tag="pair")
                        for hh in range(2):
                            h = 2 * g + hh
                            po = (h % 2) * HD
                            nc.tensor.matmul(
                                pl[0:kz, hh, 0:N],
                                lhsT=qk[po:po + HD, KC + h // 2,
                                        TCH[j][0]:TCH[j][0] + kz],
                                rhs=qk[po:po + HD, h // 2, :],
                                start=True, stop=True)
                        nc.scalar.activation(
                            out=probs[0:kz, j, 2 * g:2 * g + 2, :],
                            in_=pl[0:kz, :, 0:N], func=Exp,
                            scale=exp_scale, bias=expb[0:kz, 0:1])

                # small template-column copy so probs can be released at
                # the end of A1 (w_ts consumes this in A2 instead).
                pw_sb = actp.tile([P, 2, H, LT], BF16, tag="pwts", bufs=2)
                nc.vector.tensor_copy(
                    out=pw_sb[:, 0, :, :], in_=probs[:, 1, :, 0:LT])
                nc.vector.tensor_copy(
                    out=pw_sb[0:64, 1, :, :], in_=probs[64:128, 0, :, 0:LT])
                nc.vector.tensor_copy(
                    out=pw_sb[64:128, 1, :, :], in_=probs[0:64, 2, :, 0:LT])

                ctx = actp.tile([P, KC, N], BF16, tag="ctx", bufs=2)
                dstage = stp.tile([H, N], F32, tag="dstage")
                for h in range(H):
                    pc = ps1p.tile([HD + 1, 512], F32, tag="one")
                    nc.tensor.matmul(
                        pc[:, 0:N],
                        lhsT=v_sb[:, 0:2, h, 0:HD + 1],
                        rhs=probs[:, 0:2, h, :],
                        start=True, stop=False, perf_mode=DR)
                    nc.tensor.matmul(
                        pc[:, 0:N],
                        lhsT=v_sb[0:64, 2, h, 0:HD + 1],
                        rhs=probs[0:64, 2, h, :],
                        start=False, stop=True)
                    nc.vector.tensor_copy(
                        out=ctx[(h % 2) * HD:(h % 2) * HD + HD, h // 2, :],
                        in_=pc[0:HD, 0:N])
                    d64 = stp.tile([HD + 1, N], F32, tag="d64", bufs=2)
                    nc.any.tensor_copy(
                        out=d64[HD:HD + 1, :], in_=pc[HD:HD + 1, 0:N])
                    nc.sync.dma_start(dstage[h:h + 1, :], d64[HD:HD + 1, :])
                recf = stp.tile([H, N], F32, tag="recf")
                nc.vector.reciprocal(out=recf[:], in_=dstage[:])
                recip = stp.tile([H, N], BF16, tag="recip", bufs=2)
                nc.vector.tensor_scalar(
                    out=recip[:], in0=recf[:], scalar1=float(2.0 ** (-sv)),
                    scalar2=None, op0=MULT)
                st.update(ctx=ctx, pw_sb=pw_sb, recip=recip)
                return st

            def phase_a2_grid(s, st):
                ctx, recip, pw_sb = st["ctx"], st["recip"], st["pw_sb"]
                # normalize ctx via broadcast grid; fp8 out for proj-DR
                ctx8 = actp.tile([P, KC, N], FP8, tag="ctx8", bufs=2)
                for kc in range(KC):
                    pg = ps1p.tile([P, 512], F32, tag="one")
                    nc.tensor.matmul(pg[:, 0:N], lhsT=bexp[:, kc, :],
                                     rhs=recip[:], start=True, stop=True)
                    nc.vector.tensor_mul(
                        out=ctx8[:, kc, :], in0=ctx[:, kc, :], in1=pg[:, 0:N])
                # w_ts pieces (DVE work runs under the proj matmuls)
                prk = psp.tile([P, 2, 512], F32, tag="pair")
                for h in range(H):
                    fo = h * HD
                    nc.tensor.matmul(
                        prk[:, fo // 512, fo % 512:fo % 512 + HD],
                        lhsT=bsel[:, h, :], rhs=recip[:, 0:LT],
                        start=True, stop=True)
                prk_v = prk[:].rearrange("p a b -> p (a b)")[:, 0:H * HD] \
                    .rearrange("p (h i) -> p h i", i=HD)
                wtmp = stp.tile([P, H, LT], BF16, tag="wtmp")
                wpiece = stp.tile([P, 3, LT], F32, tag="wpiece")
                for j, (slot, po, kz) in enumerate(
                        [(1, 0, 64), (0, 0, 128), (1, 64, 64)]):
                    nc.vector.tensor_tensor(
                        out=wtmp[po:po + kz, :, :],
                        in0=pw_sb[po:po + kz, slot, :, :],
                        in1=prk_v[po:po + kz, :, :], op=MULT)
                    nc.vector.tensor_reduce(
                        out=wpiece[0:kz, j, :],
                        in_=wtmp[po:po + kz, :, :].rearrange("p h i -> p i h"),
                        axis=mybir.AxisListType.X, op=ADD)
                st["wpiece"] = wpiece
                st["ctx8"] = ctx8

            def phase_a2_wts(s, st):
                wpiece = st["wpiece"]
                pw = ps1p.tile([64, 512], F32, tag="one")
                nc.tensor.transpose(pw[:, 0:64], wpiece[0:64, 0, :],
                                    id32[0:64, 0:64])
                nc.tensor.transpose(pw[:, 64:192], wpiece[0:128, 1, :],
                                    id32[:, :])
                nc.tensor.transpose(pw[:, 192:256], wpiece[0:64, 2, :],
                                    id32[0:64, 0:64])
                wts_e = stp.tile([LT, LS], BF16, tag="wts_e")
                wden = stp.tile([LT, 1], F32, tag="wden", bufs=2)
                # w_raw carries 2^-av (from recip); undo it inside the exp
                nc.scalar.activation(out=wts_e[:], in_=pw[:, 0:LS], func=Exp,
                                     scale=float(2.0 ** sv) / H,
                                     accum_out=wden[:])
                wrec = stp.tile([LT, 1], F32, tag="wrec", bufs=2)
                nc.vector.reciprocal(out=wrec[:], in_=wden[:])
                wts = stp.tile([LT, LS], BF16, tag="wts", bufs=2)
                nc.vector.tensor_scalar_mul(wts[:], wts_e[:], wrec[:])
                wtsT = stp.tile([P, 2, LT], BF16, tag="wtsT", bufs=2)
                for mm in range(2):
                    pwt = ps1p.tile([P, 512], BF16, tag="one")
                    nc.tensor.transpose(pwt[:, 0:LT],
                                        wts[:, mm * P:(mm + 1) * P],
                                        id16[0:LT, 0:LT])
                    nc.any.tensor_copy(out=wtsT[:, mm, :], in_=pwt[:, 0:LT])
                st.update(wts=wts, wtsT=wtsT)

            def phase_b_proj(s, st):
                x_sb, ctx8 = st["x"], st["ctx8"]
                x1_sb = actp.tile([P, 3, C], BF16, tag="resid", bufs=4)
                st6 = stp.tile([P, 3, 3, 6], F32, tag="st6", bufs=2)
                mv3 = stp.tile([P, 3, 2], F32, tag="mv3", bufs=2)
                nc.vector.memset(mv3[:], 1.0)
                pdesc = float(2.0 ** (-kp))
                for tcidx, (ts, tsz) in enumerate(TCH):
                    for half in range(2):
                        pp = ps1p.tile([P, 512], F32, tag="one")
                        for g in range(KC // 2):
                            nc.tensor.matmul(
                                pp[0:tsz, 0:384],
                                lhsT=ctx8[:, 2 * g:2 * g + 2, ts:ts + tsz],
                                rhs=wproj[:, 2 * g:2 * g + 2,
                                          half * 384:(half + 1) * 384],
                                start=(g == 0), stop=(g == KC // 2 - 1),
                                perf_mode=DR)
                        nc.vector.scalar_tensor_tensor(
                            out=x1_sb[0:tsz, tcidx, half * 384:(half + 1) * 384],
                            in0=pp[0:tsz, 0:384], scalar=pdesc,
                            in1=x_sb[0:tsz, tcidx, half * 384:(half + 1) * 384],
                            op0=MULT, op1=ADD)
                    # LN2 stats interleaved per chunk
                    for sub in range(3):
                        nc.vector.bn_stats(
                            out=st6[0:tsz, tcidx, sub, :],
                            in_=x1_sb[0:tsz, tcidx, sub * 256:(sub + 1) * 256])
                    nc.vector.bn_aggr(
                        out=mv3[0:tsz, tcidx, :], in_=st6[0:tsz, tcidx, :, :])
                rstd3 = stp.tile([P, 3], F32, tag="rstd3", bufs=2)
                newton_rsqrt(rstd3, mv3[:, :, 1], 3)
                xn2_tm = actp.tile([P, 3, C], BF16, tag="xn", bufs=2)
                for tcidx, (ts, tsz) in enumerate(TCH):
                    nc.vector.tensor_scalar(
                        out=xn2_tm[0:tsz, tcidx, :],
                        in0=x1_sb[0:tsz, tcidx, :],
                        scalar1=mv3[0:tsz, tcidx, 0:1],
                        scalar2=rstd3[0:tsz, tcidx:tcidx + 1],
                        op0=SUB, op1=MULT)
                st.update(x1=x1_sb, xn2=xn2_tm)

            def phase_b_rest(s, st):
                x1_sb, xn2_tm = st["x1"], st["xn2"]
                wts, wtsT = st["wts"], st["wtsT"]
                xn2T = actp.tile([P, KC, N], FP8, tag="actT8", bufs=2)
                transpose_tm_to_fm(xn2_tm, id16, xn2T)

                h_sb = actp.tile([P, FC, N], BF16, tag="big")
                g1desc = float(2.0 ** (-k1))
                for g in range(FC // 2):
                    ph = psp.tile([P, 2, 512], F32, tag="pair")
                    for ff in range(2):
                        fcl = 2 * g + ff
                        for gg in range(KC // 2):
                            nc.tensor.matmul(
                                ph[:, ff, 0:N],
                                lhsT=w1[:, 2 * gg:2 * gg + 2,
                                        fcl * P:(fcl + 1) * P],
                                rhs=xn2T[:, 2 * gg:2 * gg + 2, :],
                                start=(gg == 0), stop=(gg == KC // 2 - 1),
                                perf_mode=DR)
                    nc.scalar.activation(
                        out=h_sb[:, 2 * g:2 * g + 2, :],
                        in_=ph[:, :, 0:N], func=Gelu, scale=g1desc)

                # MLP2 feature-major (bf16), then transpose back
                m2T = actp.tile([P, KC, N], BF16, tag="actT", bufs=2)
                for cc in range(KC):
                    pm = ps1p.tile([P, 512], F32, tag="one")
                    for fc in range(FC):
                        nc.tensor.matmul(
                            pm[:, 0:N],
                            lhsT=w2[:, fc, cc * P:(cc + 1) * P],
                            rhs=h_sb[:, fc, :],
                            start=(fc == 0), stop=(fc == FC - 1))
                    nc.any.tensor_copy(out=m2T[:, cc, :], in_=pm[:, 0:N])
                x2_sb = actp.tile([P, 3, C], BF16, tag="resid", bufs=4)
                for tcidx, (ts, tsz) in enumerate(TCH):
                    pa = ps1p.tile([P, 512], BF16, tag="one")
                    pb = ps1p.tile([P, 512], BF16, tag="one")
                    for cc in range(KC):
                        dst = pa[0:tsz, cc * P:(cc + 1) * P] if cc < 4 else \
                            pb[0:tsz, (cc - 4) * P:(cc - 3) * P]
                        nc.tensor.transpose(
                            dst, m2T[:, cc, ts:ts + tsz], id16[:, :])
                    nc.vector.tensor_add(
                        out=x2_sb[0:tsz, tcidx, 0:512],
                        in0=x1_sb[0:tsz, tcidx, 0:512], in1=pa[0:tsz, 0:512])
                    nc.vector.tensor_add(
                        out=x2_sb[0:tsz, tcidx, 512:768],
                        in0=x1_sb[0:tsz, tcidx, 512:768], in1=pb[0:tsz, 0:256])

                x2T = actp.tile([P, KC, N], BF16, tag="actT", bufs=2)
                transpose_tm_to_fm(x2_sb, id16, x2T)
                supp = actp.tile([P, 3, C], BF16, tag="vsm")
                for scidx, (ss, ssz) in enumerate(SH):
                    for half in range(2):
                        psu = ps1p.tile([P, 512], F32, tag="one")
                        for kc in range(KC):
                            nc.tensor.matmul(
                                psu[0:ssz, 0:384],
                                lhsT=x2T[:, kc, ss:ss + ssz],
                                rhs=wg[:, kc, half * 384:(half + 1) * 384],
                                start=(kc == 0), stop=(kc == KC - 1))
                        nc.any.tensor_copy(
                            out=supp[0:ssz, scidx, half * 384:(half + 1) * 384],
                            in_=psu[0:ssz, 0:384])

                for scidx, (ss, ssz) in enumerate(SH):
                    y_sb = stp.tile([P, C], F32, tag="ych", bufs=2)
                    for half in range(2):
                        py = ps1p.tile([P, 512], F32, tag="one")
                        if scidx == 0:
                            for mm in range(2):
                                nc.tensor.matmul(
                                    py[0:64, 0:384],
                                    lhsT=wtsT[:, mm, :],
                                    rhs=supp[:, mm + 1,
                                             half * 384:(half + 1) * 384],
                                    start=(mm == 0), stop=(mm == 1))
                        else:
                            nc.tensor.matmul(
                                py[0:ssz, 0:384],
                                lhsT=wts[:, (scidx - 1) * P:scidx * P],
                                rhs=supp[0:64, 0, half * 384:(half + 1) * 384],
                                start=True, stop=True)
                        nc.any.tensor_copy(
                            out=y_sb[0:ssz, half * 384:(half + 1) * 384],
                            in_=py[0:ssz, 0:384])
                    st6y = stp.tile([P, 3, 6], F32, tag="st6y", bufs=2)
                    for sub in range(3):
                        nc.vector.bn_stats(
                            out=st6y[0:ssz, sub, :],
                            in_=y_sb[0:ssz, sub * 256:(sub + 1) * 256])
                    mvy = stp.tile([P, 2], F32, tag="mvy", bufs=2)
                    nc.vector.bn_aggr(out=mvy[0:ssz, :], in_=st6y[0:ssz, :, :])
                    rstdy = stp.tile([P, 1], F32, tag="rstdy", bufs=2)
                    newton_rsqrt(rstdy[0:ssz], mvy[0:ssz, 1:2], 1)
                    yfin = stp.tile([P, C], F32, tag="yfin")
                    nc.vector.tensor_scalar(
                        out=yfin[0:ssz, :], in0=y_sb[0:ssz, :],
                        scalar1=mvy[0:ssz, 0:1], scalar2=rstdy[0:ssz, 0:1],
                        op0=SUB, op1=MULT)
                    if has_aff3:
                        nc.vector.tensor_mul(out=yfin[0:ssz, :],
                                             in0=yfin[0:ssz, :],
                                             in1=g3b[0:ssz, :])
                        nc.vector.tensor_add(out=yfin[0:ssz, :],
                                             in0=yfin[0:ssz, :],
                                             in1=b3b[0:ssz, :])
                    nc.vector.tensor_add(out=yfin[0:ssz, :],
                                         in0=yfin[0:ssz, :],
                                         in1=y_sb[0:ssz, :])
                    nc.sync.dma_start(out_e[s, ss:ss + ssz, :], yfin[0:ssz, :])

            # ---- software-pipelined emission ------------------------
            total = b_loc * repeat
            states = {}
            states[0] = phase_pre(0)
            if total > 1:
                states[1] = phase_pre(1)
            # ---- weights (in order of first use) --------------------
            wqkv = wp.tile([P, KC, 3 * C], FP8)
            for kc in range(KC):
                nc.sync.dma_start(
                    wqkv[:, kc, :],
                    wqkv_e[kc * P:(kc + 1) * P, :])
            wproj = wp.tile([P, KC, C], FP8)
            for kc in range(KC):
                nc.sync.dma_start(
                    wproj[:, kc, :], wproj_e[kc * P:(kc + 1) * P, :])
            w1 = wp.tile([P, KC, MLP_H], FP8)
            for kc in range(KC):
                nc.sync.dma_start(
                    w1[:, kc, :], w1_e[kc * P:(kc + 1) * P, :])
            w2 = wp.tile([P, FC, C], BF16)
            for fc in range(0, FC, 4):
                nc.sync.dma_start(
                    w2[:, fc:fc + 4, :],
                    w2_e[fc * P:(fc + 4) * P, :].rearrange(
                        "(fc p) m -> p fc m", p=P))
            wg = wp.tile([P, KC, C], BF16)
            for kc in range(KC):
                nc.sync.dma_start(
                    wg[:, kc, :], wg_e[kc * P:(kc + 1) * P, :])
            if has_aff3:
                g3b = wp.tile([P, C], F32)
                nc.sync.dma_start(g3b[:], g3_e[None, :].to_broadcast((P, C)))
                b3b = wp.tile([P, C], F32)
                nc.sync.dma_start(b3b[:], b3_e[None, :].to_broadcast((P, C)))

            phase_a1(0, states[0])
            if total > 1:
                phase_a1(1, states[1])
            for i in range(total):
                phase_a2_grid(i, states[i])
                phase_b_proj(i % b_loc, states[i])
                phase_a2_wts(i, states[i])
                if i + 2 < total:
                    states[i + 2] = phase_pre(i + 2)
                phase_b_rest(i % b_loc, states[i])
                del states[i]
                if i + 2 < total:
                    phase_a1(i + 2, states[i + 2])

    nc.finalize()
    return nc


def _pow2_scale(w):
    """power-of-2 exponent putting max|w| around 224 (e4m3 max 448)."""
    m = float(np.abs(w).max())
    if m <= 0:
        return 0
    return int(math.floor(math.log2(224.0 / m)))


def _preprocess(inputs):
    f32 = np.float32
    g1 = np.asarray(inputs["g1"], f32)
    b1 = np.asarray(inputs["b1"], f32)
    g2 = np.asarray(inputs["g2"], f32)
    b2 = np.asarray(inputs["b2"], f32)
    wqkv = np.asarray(inputs["Wqkv"], f32)
    w1 = np.asarray(inputs["W1"], f32)

    wqkv_eff = g1[:, None] * wqkv
    bqkv_eff = b1 @ wqkv
    scale = HD ** (-0.5)
    wqkv_eff[:, 0:C] *= scale
    bqkv_eff[0:C] *= scale

    w1_eff = g2[:, None] * w1
    bm1_eff = np.asarray(inputs["bm1"], f32) + b2 @ w1

    wproj = np.asarray(inputs["Wproj"], f32)
    bproj = np.asarray(inputs["bproj"], f32)
    bm2 = np.asarray(inputs["bm2"], f32)
    bg = np.asarray(inputs["bg"], f32)
    g3 = np.asarray(inputs["g3"], f32)
    b3 = np.asarray(inputs["b3"], f32)

    has_bias = any(
        np.abs(v).max() > 0 for v in (bqkv_eff, bm1_eff, bproj, bm2, bg))
    assert not has_bias, "fp8 kernel assumes zero biases"

    aq = _pow2_scale(wqkv_eff[:, 0:C])
    ak = _pow2_scale(wqkv_eff[:, C:2 * C])
    av = _pow2_scale(wqkv_eff[:, 2 * C:])
    k1 = _pow2_scale(w1_eff)
    kp = _pow2_scale(wproj)

    def out_scale(w):
        bound = math.sqrt(C) * float(
            np.sqrt((w.astype(np.float64) ** 2).sum(axis=0)).max())
        return int(math.floor(math.log2(224.0 / bound)))

    sv = out_scale(wqkv_eff[:, 2 * C:])

    wqkv_s = wqkv_eff.copy()
    wqkv_s[:, 0:C] *= 2.0 ** aq
    wqkv_s[:, C:2 * C] *= 2.0 ** ak
    wqkv_s[:, 2 * C:] *= 2.0 ** av

    bexp = np.zeros((H, C), f32)
    for h in range(H):
        bexp[h, h * HD:(h + 1) * HD] = 1.0
    bsel = np.kron(np.eye(H, dtype=f32), np.ones((1, P), f32))

    bf = ml_dtypes.bfloat16
    f8 = ml_dtypes.float8_e4m3
    wm = {
        "wqkv8": wqkv_s.astype(f8),
        "wproj8": (wproj * 2.0 ** kp).astype(f8),
        "w18": (w1_eff * 2.0 ** k1).astype(f8),
        "w2": np.asarray(inputs["W2"], f32).astype(bf),
        "wg": np.asarray(inputs["Wg"], f32).astype(bf),
        "bexp": bexp.astype(bf),
        "bsel": bsel.astype(bf),
        "id32": np.eye(P, dtype=f32),
        "id16": np.eye(P, dtype=f32).astype(bf),
    }
    flags = {"scales": (aq, ak, av, k1, kp, sv)}
    has_aff3 = bool(np.abs(g3 - 1.0).max() > 0 or np.abs(b3).max() > 0)
    flags["has_aff3"] = has_aff3
    if has_aff3:
        wm["g3v"] = g3
        wm["b3v"] = b3
    return wm, flags


_CACHE = {}


def _get_nc(flags, b_loc=B_LOC):
    key = (tuple(sorted((k, v) for k, v in flags.items())), b_loc)
    if key not in _CACHE:
        _CACHE[key] = build_nc(b_loc=b_loc, flags=flags)
    return _CACHE[key]


def kernel(**inputs) -> np.ndarray:
    from concourse.bass_utils import run_bass_kernel_spmd

    x = np.ascontiguousarray(
        np.asarray(inputs["x"], np.float32)).astype(ml_dtypes.bfloat16)
    wm, flags = _preprocess(inputs)
    nc = _get_nc(flags)

    in_maps = []
    for i in range(N_CORES):
        m = dict(wm)
        m["x"] = np.ascontiguousarray(x[i * B_LOC:(i + 1) * B_LOC])
        in_maps.append(m)

    res = run_bass_kernel_spmd(nc, in_maps, core_ids=list(range(N_CORES)))
    out = np.concatenate([res.results[i]["out"] for i in range(N_CORES)], axis=0)
    return out.astype(np.float32)
